# revision 12
# baseline (speedup 1.0000x reference)
"""Trainium2 Bass kernel for nn_Batched_STHD_SpGAT (gnn_message_passing).

Computes, on 8 NeuronCores (SPMD, node-sharded):
  ll_prot = sum(P_sub * F_c)/n           (Gaussian log-likelihood term)
  ce_space = -sum(P[src]*alpha*log(P[dst]+eps))/n   (GATv2 edge-softmax CE)
  P_sub = softmax(W_param[subset_idx], axis=1)

Sharding: nodes (and their incoming edges) are partitioned across 8 cores.
Launch A computes, per core, the Gaussian term F, P = softmax(W), the GAT
projections xl/xr and the ll partial, and emits a per-node record table
[xl | P].  The record tables are exchanged/gathered into per-edge-slot
records (destination-ordered slot table, all cores' records concatenated),
and launch B computes the edge softmax + CE partials on-device.

Algebra on device:
  F[n,c] = -0.5*sum_g (x-Mu*S)^2/Var
         = sum_g x^2 * (-0.5/Var)  + S * sum_g x*Mu/Var  - 0.5*S^2*sum_g Mu^2/Var
  The -0.5*S^2*d term is folded into the x^2 matmul via an extra "gene" row
  (x2 row = S^2, A row = -0.5*d); the b_l/b_r biases are folded into the
  xl/xr matmul via a ones row of x.
  Edge softmax is computed without the segment-max shift (scores are O(1)).
"""

import sys

sys.path.insert(0, "/opt/trn_rl_repo")

import numpy as np
import ml_dtypes

import concourse.bacc as bacc
import concourse.bass as bass
import concourse.tile as tile
from concourse import mybir
from concourse.bass_utils import run_bass_kernel_spmd

F32 = mybir.dt.float32
BF16 = mybir.dt.bfloat16
AF = mybir.ActivationFunctionType
ALU = mybir.AluOpType
AX = mybir.AxisListType

# problem constants (hardcoded per contest contract)
NCORES = 8
N = 10000           # nodes total
NPC = N // NCORES   # real nodes per core (1250)
T = 10              # node tiles per core
NM = 128 * T        # padded nodes per core (1280)
NMG = NM * NCORES   # padded global nodes (10240)
G = 500             # genes
GP = 512            # padded genes (4 chunks of 128)
C = 17              # classes
CP = 32             # padded classes
H = 8               # GAT hidden
REC = 26            # record row width: [xl(8) | P(17) | pad]
NEG_SLOPE = 0.2
WPAD = -100.0       # pad-class logit

_prog_cache = {}


# --------------------------------------------------------------------------
# launch A: Gaussian term, softmax, GAT projections, ll partial, records
# --------------------------------------------------------------------------
def build_program_a():
    nc = bacc.Bacc("TRN2", target_bir_lowering=False, debug=False,
                   num_devices=NCORES)

    d_xtb = nc.dram_tensor("xtb", [GP, NM], BF16, kind="ExternalInput")
    d_wnode = nc.dram_tensor("wnode", [128, T * CP], F32, kind="ExternalInput")
    d_scol = nc.dram_tensor("scol", [128, T], F32, kind="ExternalInput")
    d_srow = nc.dram_tensor("srow", [1, NM], F32, kind="ExternalInput")
    d_mut = nc.dram_tensor("mut", [GP, CP], F32, kind="ExternalInput")
    d_vart = nc.dram_tensor("vart", [GP, CP], F32, kind="ExternalInput")
    d_wl = nc.dram_tensor("wl", [GP, H], F32, kind="ExternalInput")
    d_wr = nc.dram_tensor("wr", [GP, H], F32, kind="ExternalInput")
    d_ident = nc.dram_tensor("ident", [128, 128], F32, kind="ExternalInput")

    d_pout = nc.dram_tensor("pout", [128, T * CP], F32, kind="ExternalOutput")
    d_lout = nc.dram_tensor("lout", [128, T * CP], F32, kind="ExternalOutput")
    d_xrout = nc.dram_tensor("xrout", [128, T * H], F32, kind="ExternalOutput")
    d_recout = nc.dram_tensor("recout", [NM, REC], F32, kind="ExternalOutput")
    d_scal = nc.dram_tensor("scal", [128, 1], F32, kind="ExternalOutput")

    with tile.TileContext(nc) as tc:
        with (
            tc.tile_pool(name="sb", bufs=1) as sb,
            tc.tile_pool(name="dramp", bufs=1, space="DRAM") as dpool,
        ):
            # ======== loads ========
            xtb = sb.tile([128, 4, NM], BF16)
            nc.sync.dma_start(xtb[:], d_xtb.ap().rearrange("(k p) n -> p k n", p=128))
            wnode = sb.tile([128, T, CP], F32)
            nc.sync.dma_start(wnode[:], d_wnode.ap())
            scol = sb.tile([128, T], F32)
            nc.sync.dma_start(scol[:], d_scol.ap())
            srow = sb.tile([1, NM], F32)
            nc.sync.dma_start(srow[:], d_srow.ap())
            mut = sb.tile([128, 4, CP], F32)
            nc.sync.dma_start(mut[:], d_mut.ap().rearrange("(k p) c -> p k c", p=128))
            vart = sb.tile([128, 4, CP], F32)
            nc.sync.dma_start(vart[:], d_vart.ap().rearrange("(k p) c -> p k c", p=128))
            wlt = sb.tile([128, 4, H], F32)
            nc.sync.dma_start(wlt[:], d_wl.ap().rearrange("(k p) h -> p k h", p=128))
            wrt = sb.tile([128, 4, H], F32)
            nc.sync.dma_start(wrt[:], d_wr.ap().rearrange("(k p) h -> p k h", p=128))
            ident = sb.tile([128, 128], F32)
            nc.sync.dma_start(ident[:], d_ident.ap())

            # ======== weight prep ========
            recipv = sb.tile([128, 4, CP], F32)
            nc.vector.reciprocal(recipv[:], vart[:])
            btf = sb.tile([128, 4, CP], F32)          # B = Mu/Var (f32)
            nc.vector.tensor_tensor(btf[:], mut[:], recipv[:], op=ALU.mult)
            wcat = sb.tile([128, 4, CP + 2 * H], BF16)
            nc.vector.tensor_copy(wcat[:, :, 0:CP], btf[:])
            nc.vector.tensor_copy(wcat[:, :, CP:CP + H], wlt[:])
            nc.vector.tensor_copy(wcat[:, :, CP + H:CP + 2 * H], wrt[:])
            acat = sb.tile([128, 4, CP], BF16)        # -0.5/Var
            nc.scalar.mul(acat[:], recipv[:], -0.5)
            m2v = sb.tile([128, 4, CP], F32)
            nc.vector.tensor_tensor(m2v[:], mut[:], btf[:], op=ALU.mult)
            ones = sb.tile([128, 1], F32)
            nc.vector.memset(ones[:], 1.0)
            with tc.tile_pool(name="psd", bufs=1, space="PSUM") as psd:
                dps = psd.tile([1, CP], F32)
                for k in range(4):
                    nc.tensor.matmul(
                        dps[:], ones[:], m2v[:, k, :], start=(k == 0), stop=(k == 3)
                    )
                drow = sb.tile([1, CP], BF16)
                nc.scalar.mul(drow[:], dps[:], -0.5)   # -0.5*d, cast bf16
            # patch Acat gene-row 500 (chunk 3, partition 116) via DRAM bounce
            drow_d = dpool.tile([1, CP], BF16)
            nc.sync.dma_start(drow_d[:], drow[:])
            nc.sync.dma_start(acat[116:117, 3, :], drow_d[:])

            # x^2 (bf16); gene-row 500 patched with S^2
            x2b = sb.tile([128, 4, NM], BF16)
            nc.vector.tensor_tensor(
                x2b[:, 0:3, :], xtb[:, 0:3, :], xtb[:, 0:3, :], op=ALU.mult
            )
            nc.scalar.activation(x2b[:, 3, :], xtb[:, 3, :], AF.Square)
            s2row = sb.tile([1, NM], BF16)
            nc.scalar.activation(s2row[:], srow[:], AF.Square)
            s2row_d = dpool.tile([1, NM], BF16)
            nc.sync.dma_start(s2row_d[:], s2row[:])
            nc.sync.dma_start(x2b[116:117, 3, :], s2row_d[:])

            # ======== main matmuls (class-major, stationary weights) ========
            f1 = sb.tile([CP + 2 * H, NM], F32)
            q = sb.tile([CP, NM], F32)
            with tc.tile_pool(name="psm", bufs=1, space="PSUM") as psm:
                nchunks = [(0, 512), (512, 512), (1024, 256)]
                o1ps = [psm.tile([CP + 2 * H, w], F32, name=f"o1ps{i}")
                        for i, (b, w) in enumerate(nchunks)]
                qps = [psm.tile([CP, w], F32, name=f"qps{i}")
                       for i, (b, w) in enumerate(nchunks)]
                for i, (b, w) in enumerate(nchunks):
                    for k in range(4):
                        nc.tensor.matmul(
                            o1ps[i][:], wcat[:, k, :], xtb[:, k, b:b + w],
                            start=(k == 0), stop=(k == 3),
                        )
                    for k in range(4):
                        nc.tensor.matmul(
                            qps[i][:], acat[:, k, :], x2b[:, k, b:b + w],
                            start=(k == 0), stop=(k == 3),
                        )
                for i, (b, w) in enumerate(nchunks):
                    nc.vector.tensor_copy(f1[:, b:b + w], o1ps[i][:])
                    nc.scalar.copy(q[:, b:b + w], qps[i][:])

            # ======== transpose to node-major ========
            fnq = sb.tile([128, T, 80], F32)
            with tc.tile_pool(name="pst", bufs=1, space="PSUM") as pst:
                tra = pst.tile([128, 6 * 80], F32, name="tra")
                trb = pst.tile([128, 4 * 80], F32, name="trb")
                for t in range(T):
                    dst = tra if t < 6 else trb
                    off = (t if t < 6 else t - 6) * 80
                    nc.tensor.transpose(
                        dst[:, off:off + 48], f1[:, 128 * t:128 * (t + 1)],
                        ident[0:CP + 2 * H, 0:CP + 2 * H],
                    )
                    nc.tensor.transpose(
                        dst[:, off + 48:off + 80], q[:, 128 * t:128 * (t + 1)],
                        ident[0:CP, 0:CP],
                    )
                nc.vector.tensor_copy(fnq[:, 0:6, :], tra[:])
                nc.vector.tensor_copy(fnq[:, 6:10, :], trb[:])

            # ======== F (node-major) ========
            scol_bc = scol[:].unsqueeze(2).broadcast_to([128, T, CP])
            fmat = sb.tile([128, T, CP], F32)
            nc.vector.tensor_tensor(fmat[:], fnq[:, :, 0:CP], scol_bc, op=ALU.mult)
            nc.vector.tensor_tensor(fmat[:], fmat[:], fnq[:, :, 48:80], op=ALU.add)

            # ======== P softmax (node-major, |W| < 1 so no max shift) ========
            expw = sb.tile([128, T, CP], F32)
            nc.scalar.activation(expw[:], wnode[:], AF.Exp)
            rs = sb.tile([128, T], F32)
            nc.vector.tensor_reduce(rs[:], expw[:], axis=AX.X, op=ALU.add)
            rr = sb.tile([128, T], F32)
            nc.vector.reciprocal(rr[:], rs[:])
            pmat = sb.tile([128, T, CP], F32)
            nc.vector.tensor_tensor(
                pmat[:], expw[:], rr[:].unsqueeze(2).broadcast_to([128, T, CP]),
                op=ALU.mult,
            )
            lmat = sb.tile([128, T, CP], F32)
            peps = sb.tile([128, T, CP], F32)
            nc.vector.tensor_scalar(peps[:], pmat[:], 1e-8, None, op0=ALU.add)
            nc.scalar.activation(lmat[:], peps[:], AF.Ln)

            # ll partial (per-partition; host sums the 128 values)
            pf = sb.tile([128, T, CP], F32)
            nc.vector.tensor_tensor(pf[:], pmat[:], fmat[:], op=ALU.mult)
            llv = sb.tile([128, 1], F32)
            nc.vector.tensor_reduce(llv[:], pf[:], axis=AX.XY, op=ALU.add)
            nc.sync.dma_start(d_scal.ap(), llv[:])

            # ======== per-node record table [xl | P] ========
            rec = sb.tile([128, T, REC], F32)
            nc.vector.memset(rec[:, :, H + C:REC], 0.0)
            nc.vector.tensor_copy(rec[:, :, 0:H], fnq[:, :, CP:CP + H])   # xl
            nc.vector.tensor_copy(rec[:, :, H:H + C], pmat[:, :, 0:C])    # P
            nc.sync.dma_start(
                d_recout.ap().rearrange("(t p) c -> p t c", p=128), rec[:]
            )
            nc.sync.dma_start(d_xrout.ap(),
                              fnq[:, :, CP + H:CP + 2 * H])
            nc.sync.dma_start(d_lout.ap(), lmat[:])
            nc.sync.dma_start(d_pout.ap(), pmat[:])

    nc.compile()
    return nc


# --------------------------------------------------------------------------
# launch B: edge softmax + CE partial (node-major slot table)
# --------------------------------------------------------------------------
def build_program_b(D: int):
    S = 128 * T * D
    nc = bacc.Bacc("TRN2", target_bir_lowering=False, debug=False,
                   num_devices=NCORES)

    d_recg = nc.dram_tensor("recg", [128, T * D * REC], F32, kind="ExternalInput")
    d_xr = nc.dram_tensor("xr", [128, T * H], F32, kind="ExternalInput")
    d_lmat = nc.dram_tensor("lmat", [128, T * CP], F32, kind="ExternalInput")
    d_attb = nc.dram_tensor("attb", [128, H], F32, kind="ExternalInput")
    d_smask = nc.dram_tensor("smask", [128, T * D], F32, kind="ExternalInput")
    d_scal = nc.dram_tensor("scal", [128, 1], F32, kind="ExternalOutput")

    with tile.TileContext(nc) as tc:
        with tc.tile_pool(name="sb", bufs=1) as sb:
            recg = sb.tile([128, T * D, REC], F32)
            nc.sync.dma_start(recg[:], d_recg.ap())
            xr = sb.tile([128, T, H], F32)
            nc.sync.dma_start(xr[:], d_xr.ap())
            lmat = sb.tile([128, T, CP], F32)
            nc.sync.dma_start(lmat[:], d_lmat.ap())
            attb = sb.tile([128, H], F32)
            nc.sync.dma_start(attb[:], d_attb.ap())
            smask = sb.tile([128, T, D], F32)
            nc.sync.dma_start(smask[:], d_smask.ap())

            recg4 = recg[:].rearrange("p (t d) r -> p t d r", d=D)
            xr_bc = xr[:].unsqueeze(2).broadcast_to([128, T, D, H])
            h = sb.tile([128, T, D, H], F32)
            nc.vector.tensor_tensor(h[:], recg4[:, :, :, 0:H], xr_bc, op=ALU.add)
            lr = sb.tile([128, T, D, H], F32)
            hs = sb.tile([128, T, D, H], F32)
            nc.scalar.mul(hs[:], h[:], NEG_SLOPE)
            nc.vector.tensor_tensor(lr[:], h[:], hs[:], op=ALU.max)
            ez = sb.tile([128, T, D, H], F32)
            att_bc = attb[:].unsqueeze(1).unsqueeze(1).broadcast_to([128, T, D, H])
            nc.vector.tensor_tensor(ez[:], lr[:], att_bc, op=ALU.mult)
            e = sb.tile([128, T, D], F32)
            nc.vector.tensor_reduce(e[:], ez[:], axis=AX.X, op=ALU.add)
            ex = sb.tile([128, T, D], F32)
            nc.scalar.activation(ex[:], e[:], AF.Exp)
            exm = sb.tile([128, T, D], F32)
            nc.vector.tensor_tensor(exm[:], ex[:], smask[:], op=ALU.mult)
            den = sb.tile([128, T], F32)
            nc.vector.tensor_reduce(den[:], exm[:], axis=AX.X, op=ALU.add)
            nc.vector.tensor_scalar(den[:], den[:], 1e-30, None, op0=ALU.add)
            rden = sb.tile([128, T], F32)
            nc.vector.reciprocal(rden[:], den[:])
            alpha = sb.tile([128, T, D], F32)
            nc.vector.tensor_tensor(
                alpha[:], exm[:], rden[:].unsqueeze(2).broadcast_to([128, T, D]),
                op=ALU.mult,
            )
            kz = sb.tile([128, T, D, C], F32)
            l_bc = lmat[:, :, 0:C].unsqueeze(2).broadcast_to([128, T, D, C])
            nc.vector.tensor_tensor(kz[:], recg4[:, :, :, H:H + C], l_bc,
                                    op=ALU.mult)
            kv = sb.tile([128, T, D], F32)
            nc.vector.tensor_reduce(kv[:], kz[:], axis=AX.X, op=ALU.add)
            ak = sb.tile([128, T, D], F32)
            nc.vector.tensor_tensor(ak[:], alpha[:], kv[:], op=ALU.mult)
            cev = sb.tile([128, 1], F32)
            nc.vector.tensor_reduce(cev[:], ak[:], axis=AX.XY, op=ALU.add)
            nc.sync.dma_start(d_scal.ap(), cev[:])

    nc.compile()
    return nc


# --------------------------------------------------------------------------
# host-side sharding
# --------------------------------------------------------------------------
def shard_inputs(x_sub, Mu, Var, edge_index_sub, subset_idx,
                 W_param, S_param, W_l, b_l, W_r, b_r, att):
    x_sub = np.asarray(x_sub, np.float32)
    Mu = np.asarray(Mu, np.float32)
    Var = np.asarray(Var, np.float32)
    ei = np.asarray(edge_index_sub).astype(np.int64)
    sub = np.asarray(subset_idx).astype(np.int64)
    W_param = np.asarray(W_param, np.float32)
    S_param = np.asarray(S_param, np.float32)
    W_l = np.asarray(W_l, np.float32)
    W_r = np.asarray(W_r, np.float32)
    b_l = np.asarray(b_l, np.float32)
    b_r = np.asarray(b_r, np.float32)
    att = np.asarray(att, np.float32)

    src, dst = ei[0], ei[1]
    deg = np.bincount(dst, minlength=N)
    D = max(int(deg.max()), 1)

    mut = np.zeros((GP, CP), np.float32)
    mut[:G, :C] = Mu.T
    vart = np.ones((GP, CP), np.float32)
    vart[:G, :C] = Var.T
    vart[G + 1, :] = 1e30          # ones-row of x^2 must not contribute
    wl = np.zeros((GP, H), np.float32)
    wl[:G] = W_l
    wl[G + 1] = b_l                # bias via ones-row of x
    wr = np.zeros((GP, H), np.float32)
    wr[:G] = W_r
    wr[G + 1] = b_r
    attb = np.tile(att[None, :], (128, 1)).astype(np.float32)
    ident = np.eye(128, dtype=np.float32)

    Wsub_all = W_param[sub]        # host row-gather, per sharding hint
    Ssub_all = S_param[sub, 0]

    in_maps_a = []
    aux = []
    for m in range(NCORES):
        lo, hi = m * NPC, (m + 1) * NPC
        xtb = np.zeros((GP, NM), np.float32)
        xtb[:G, :NPC] = x_sub[lo:hi].T
        xtb[G + 1, :NPC] = 1.0
        wn = np.full((NM, CP), WPAD, np.float32)
        wn[:NPC, :C] = Wsub_all[lo:hi]
        wn[NPC:, :] = 0.0
        wnode = wn.reshape(T, 128, CP).transpose(1, 0, 2).reshape(128, T * CP)
        sv = np.zeros(NM, np.float32)
        sv[:NPC] = Ssub_all[lo:hi]
        scol = sv.reshape(T, 128).T.copy()
        srow = sv[None, :]
        # edges of this core, slotted by local destination
        sel = (dst >= lo) & (dst < hi)
        sm, dm = src[sel], dst[sel] - lo
        order = np.argsort(dm, kind="stable")
        ds, ss = dm[order], sm[order]
        seg_start = np.searchsorted(ds, ds)
        slot = np.arange(len(ds)) - seg_start
        slot_src = np.zeros((NM, D), np.int64)
        slot_mask = np.zeros((NM, D), np.float32)
        gid = (ss // NPC) * NM + (ss % NPC)   # padded global node id
        slot_src[ds, slot] = gid
        slot_mask[ds, slot] = 1.0
        # node-major [p, t, d]
        sgid = slot_src.reshape(T, 128, D).transpose(1, 0, 2).reshape(128, T * D)
        smask = slot_mask.reshape(T, 128, D).transpose(1, 0, 2).reshape(128, T * D)
        in_maps_a.append({
            "xtb": xtb.astype(ml_dtypes.bfloat16),
            "wnode": np.ascontiguousarray(wnode),
            "scol": np.ascontiguousarray(scol),
            "srow": np.ascontiguousarray(srow),
            "mut": mut, "vart": vart, "wl": wl, "wr": wr,
            "ident": ident,
        })
        aux.append({
            "sgid": sgid,
            "smask": np.ascontiguousarray(smask),
            "attb": attb,
        })
    return in_maps_a, aux, D


def unshard_p(results_a):
    pparts = []
    for r in results_a:
        p = np.asarray(r["pout"]).reshape(128, T, CP).transpose(1, 0, 2)
        pparts.append(p.reshape(NM, CP)[:NPC, :C])
    return np.concatenate(pparts, axis=0).astype(np.float32)


# --------------------------------------------------------------------------
# entry point
# --------------------------------------------------------------------------
_last_res = None


def kernel(**inputs):
    global _last_res
    in_maps_a, aux, D = shard_inputs(**inputs)
    if "A" not in _prog_cache:
        _prog_cache["A"] = build_program_a()
    if ("B", D) not in _prog_cache:
        _prog_cache[("B", D)] = build_program_b(D)

    res_a = run_bass_kernel_spmd(_prog_cache["A"], in_maps_a,
                                 list(range(NCORES)))
    # exchange: concatenate record tables, gather per-edge-slot records
    rec_all = np.concatenate(
        [np.asarray(r["recout"], np.float32) for r in res_a.results], axis=0
    )
    in_maps_b = []
    for m in range(NCORES):
        recg = rec_all[aux[m]["sgid"].reshape(-1)].reshape(128, T * D * REC)
        in_maps_b.append({
            "recg": np.ascontiguousarray(recg),
            "xr": np.asarray(res_a.results[m]["xrout"], np.float32),
            "lmat": np.asarray(res_a.results[m]["lout"], np.float32),
            "attb": aux[m]["attb"],
            "smask": aux[m]["smask"],
        })
    res_b = run_bass_kernel_spmd(_prog_cache[("B", D)], in_maps_b,
                                 list(range(NCORES)))
    _last_res = (res_a, res_b)

    ll = sum(float(np.asarray(r["scal"], np.float64).sum()) for r in res_a.results)
    ce = sum(float(np.asarray(r["scal"], np.float64).sum()) for r in res_b.results)
    P = unshard_p(res_a.results)
    return np.float32(ll / N), np.float32(-ce / N), P


if __name__ == "__main__":
    import reference

    inp = reference.setup_inputs()
    inp = {k: np.asarray(v) for k, v in inp.items()}
    out = kernel(**inp)
    print("ll", out[0], "ce", out[1], "P", out[2].shape)


# revision 13
# speedup vs baseline: 1.0207x; 1.0207x over previous
"""Trainium2 Bass kernel for nn_Batched_STHD_SpGAT (gnn_message_passing).

Computes, on 8 NeuronCores (SPMD, node-sharded):
  ll_prot = sum(P_sub * F_c)/n           (Gaussian log-likelihood term)
  ce_space = -sum(P[src]*alpha*log(P[dst]+eps))/n   (GATv2 edge-softmax CE)
  P_sub = softmax(W_param[subset_idx], axis=1)

Sharding: nodes (and their incoming edges) are partitioned across 8 cores.
Launch A computes, per core, the Gaussian term F, P = softmax(W), the GAT
projections xl/xr and the ll partial, and emits a per-node record table
[xl | P].  The record tables are exchanged/gathered into per-edge-slot
records (destination-ordered slot table, all cores' records concatenated),
and launch B computes the edge softmax + CE partials on-device.

Algebra on device:
  F[n,c] = -0.5*sum_g (x-Mu*S)^2/Var
         = sum_g x^2 * (-0.5/Var)  + S * sum_g x*Mu/Var  - 0.5*S^2*sum_g Mu^2/Var
  Gene-row 500 of x holds S (so x^2 row 500 is S^2 automatically) and the A
  matrix row 500 is patched to -0.5*d (d = sum_g Mu^2/Var), folding the
  -0.5*S^2*d term into the x^2 matmul; the b_l/b_r biases are folded into
  the xl/xr matmul via a ones row of x (row 501).
  Edge softmax is computed without the segment-max shift (scores are O(1)).
"""

import sys

sys.path.insert(0, "/opt/trn_rl_repo")

import numpy as np
import ml_dtypes

import concourse.bacc as bacc
import concourse.tile as tile
from concourse import mybir
from concourse.bass_utils import run_bass_kernel_spmd

F32 = mybir.dt.float32
BF16 = mybir.dt.bfloat16
AF = mybir.ActivationFunctionType
ALU = mybir.AluOpType
AX = mybir.AxisListType

# problem constants (hardcoded per contest contract)
NCORES = 8
N = 10000           # nodes total
NPC = N // NCORES   # real nodes per core (1250)
T = 10              # node tiles per core
NM = 128 * T        # padded nodes per core (1280)
NMG = NM * NCORES   # padded global nodes (10240)
G = 500             # genes
GP = 512            # padded genes (4 chunks of 128)
C = 17              # classes
CP = 32             # padded classes
H = 8               # GAT hidden
REC = 26            # record row width: [xl(8) | P(17) | pad]
NEG_SLOPE = 0.2
WPAD = -100.0       # pad-class logit

# packed small-input column offsets for launch A ([128, APK] f32):
#  wnode [T*CP] | scol [T] | ident [128] | mut [4*CP] | vart [4*CP]
#  | wl [4*H] | wr [4*H]
A_WN, A_SC, A_ID = 0, T * CP, T * CP + T
A_MU = A_ID + 128
A_VA = A_MU + 4 * CP
A_WL = A_VA + 4 * CP
A_WR = A_WL + 4 * H
APK = A_WR + 4 * H
# packed small-input column offsets for launch B ([128, BPK(D)] f32):
#  xr [T*H] | lmat [T*CP] | attb [H] | smask [T*D]
B_XR, B_LM, B_AT, B_SM = 0, T * H, T * H + T * CP, T * H + T * CP + H

_prog_cache = {}


# --------------------------------------------------------------------------
# launch A: Gaussian term, softmax, GAT projections, ll partial, records
# --------------------------------------------------------------------------
def build_program_a():
    nc = bacc.Bacc("TRN2", target_bir_lowering=False, debug=False,
                   num_devices=NCORES)

    d_xtb = nc.dram_tensor("xtb", [GP, NM], BF16, kind="ExternalInput")
    d_pk = nc.dram_tensor("pk", [128, APK], F32, kind="ExternalInput")

    d_pout = nc.dram_tensor("pout", [128, T * CP], F32, kind="ExternalOutput")
    d_lout = nc.dram_tensor("lout", [128, T * CP], F32, kind="ExternalOutput")
    d_xrout = nc.dram_tensor("xrout", [128, T * H], F32, kind="ExternalOutput")
    d_recout = nc.dram_tensor("recout", [NM, REC], F32, kind="ExternalOutput")
    d_scal = nc.dram_tensor("scal", [128, 1], F32, kind="ExternalOutput")

    with tile.TileContext(nc) as tc:
        with (
            tc.tile_pool(name="sb", bufs=1) as sb,
            tc.tile_pool(name="dramp", bufs=1, space="DRAM") as dpool,
        ):
            # ======== loads ========
            xtb = sb.tile([128, 4, NM], BF16)
            nc.sync.dma_start(xtb[:], d_xtb.ap().rearrange("(k p) n -> p k n", p=128))
            pk = sb.tile([128, APK], F32)
            nc.sync.dma_start(pk[:], d_pk.ap())
            wnode = pk[:, A_WN:A_WN + T * CP].rearrange("p (t c) -> p t c", c=CP)
            scol = pk[:, A_SC:A_SC + T]
            ident = pk[:, A_ID:A_ID + 128]
            mut = pk[:, A_MU:A_MU + 4 * CP].rearrange("p (k c) -> p k c", c=CP)
            vart = pk[:, A_VA:A_VA + 4 * CP].rearrange("p (k c) -> p k c", c=CP)
            wlt = pk[:, A_WL:A_WL + 4 * H].rearrange("p (k h) -> p k h", h=H)
            wrt = pk[:, A_WR:A_WR + 4 * H].rearrange("p (k h) -> p k h", h=H)

            # ======== weight prep ========
            recipv = sb.tile([128, 4, CP], F32)
            nc.vector.reciprocal(recipv[:], vart)
            btf = sb.tile([128, 4, CP], F32)          # B = Mu/Var (f32)
            nc.vector.tensor_tensor(btf[:], mut, recipv[:], op=ALU.mult)
            wcat = sb.tile([128, 4, CP + 2 * H], BF16)
            nc.vector.tensor_copy(wcat[:, :, 0:CP], btf[:])
            nc.vector.tensor_copy(wcat[:, :, CP:CP + H], wlt)
            nc.vector.tensor_copy(wcat[:, :, CP + H:CP + 2 * H], wrt)
            acat = sb.tile([128, 4, CP], BF16)        # -0.5/Var
            nc.scalar.mul(acat[:], recipv[:], -0.5)
            m2v = sb.tile([128, 4, CP], F32)
            nc.vector.tensor_tensor(m2v[:], mut, btf[:], op=ALU.mult)
            ones = sb.tile([128, 1], F32)
            nc.vector.memset(ones[:], 1.0)
            with tc.tile_pool(name="psd", bufs=1, space="PSUM") as psd:
                dps = psd.tile([1, CP], F32)
                for k in range(4):
                    nc.tensor.matmul(
                        dps[:], ones[:], m2v[:, k, :], start=(k == 0), stop=(k == 3)
                    )
                drow = sb.tile([1, CP], BF16)
                nc.scalar.mul(drow[:], dps[:], -0.5)   # -0.5*d, cast bf16
            # patch Acat gene-row 500 (chunk 3, partition 116) via DRAM bounce
            drow_d = dpool.tile([1, CP], BF16)
            nc.sync.dma_start(drow_d[:], drow[:])
            nc.sync.dma_start(acat[116:117, 3, :], drow_d[:])

            # x^2 (bf16); x row 500 = S (host), so x^2 row 500 = S^2
            x2b = sb.tile([128, 4, NM], BF16)
            nc.vector.tensor_tensor(
                x2b[:, 0:3, :], xtb[:, 0:3, :], xtb[:, 0:3, :], op=ALU.mult
            )
            nc.scalar.activation(x2b[:, 3, :], xtb[:, 3, :], AF.Square)

            # ======== main matmuls (class-major, stationary weights) ========
            f1 = sb.tile([CP + 2 * H, NM], F32)
            q = sb.tile([CP, NM], F32)
            with tc.tile_pool(name="psm", bufs=1, space="PSUM") as psm:
                o1ps = psm.tile([CP + 2 * H, NM], F32)
                qps = psm.tile([CP, NM], F32)
                for b, w in [(0, 512), (512, 512), (1024, 256)]:
                    for k in range(4):
                        nc.tensor.matmul(
                            o1ps[:, b:b + w], wcat[:, k, :], xtb[:, k, b:b + w],
                            start=(k == 0), stop=(k == 3),
                        )
                    for k in range(4):
                        nc.tensor.matmul(
                            qps[:, b:b + w], acat[:, k, :], x2b[:, k, b:b + w],
                            start=(k == 0), stop=(k == 3),
                        )
                nc.vector.tensor_copy(f1[:], o1ps[:])
                nc.scalar.copy(q[:], qps[:])

            # ======== transpose to node-major ========
            fnq = sb.tile([128, T, 80], F32)
            with tc.tile_pool(name="pst", bufs=1, space="PSUM") as pst:
                tra = pst.tile([128, 6 * 80], F32, name="tra")
                trb = pst.tile([128, 4 * 80], F32, name="trb")
                for t in range(T):
                    dst = tra if t < 6 else trb
                    off = (t if t < 6 else t - 6) * 80
                    nc.tensor.transpose(
                        dst[:, off:off + 48], f1[:, 128 * t:128 * (t + 1)],
                        ident[0:CP + 2 * H, 0:CP + 2 * H],
                    )
                    nc.tensor.transpose(
                        dst[:, off + 48:off + 80], q[:, 128 * t:128 * (t + 1)],
                        ident[0:CP, 0:CP],
                    )
                nc.vector.tensor_copy(fnq[:, 0:6, :], tra[:])
                nc.vector.tensor_copy(fnq[:, 6:10, :], trb[:])

            # ======== F (node-major) ========
            scol_bc = scol.unsqueeze(2).broadcast_to([128, T, CP])
            fmat = sb.tile([128, T, CP], F32)
            nc.vector.tensor_tensor(fmat[:], fnq[:, :, 0:CP], scol_bc, op=ALU.mult)
            nc.vector.tensor_tensor(fmat[:], fmat[:], fnq[:, :, 48:80], op=ALU.add)

            # ======== P softmax (node-major, |W| < 1 so no max shift) ========
            expw = sb.tile([128, T, CP], F32)
            nc.scalar.activation(expw[:], wnode, AF.Exp)
            rs = sb.tile([128, T], F32)
            nc.vector.tensor_reduce(rs[:], expw[:], axis=AX.X, op=ALU.add)
            rr = sb.tile([128, T], F32)
            nc.vector.reciprocal(rr[:], rs[:])
            pmat = sb.tile([128, T, CP], F32)
            nc.vector.tensor_tensor(
                pmat[:], expw[:], rr[:].unsqueeze(2).broadcast_to([128, T, CP]),
                op=ALU.mult,
            )
            lmat = sb.tile([128, T, CP], F32)
            peps = sb.tile([128, T, CP], F32)
            nc.vector.tensor_scalar(peps[:], pmat[:], 1e-8, None, op0=ALU.add)
            nc.scalar.activation(lmat[:], peps[:], AF.Ln)

            # ll partial (per-partition; host sums the 128 values)
            pf = sb.tile([128, T, CP], F32)
            nc.vector.tensor_tensor(pf[:], pmat[:], fmat[:], op=ALU.mult)
            llv = sb.tile([128, 1], F32)
            nc.vector.tensor_reduce(llv[:], pf[:], axis=AX.XY, op=ALU.add)
            nc.sync.dma_start(d_scal.ap(), llv[:])

            # ======== per-node record table [xl | P] ========
            rec = sb.tile([128, T, REC], F32)
            nc.vector.memset(rec[:, :, H + C:REC], 0.0)
            nc.vector.tensor_copy(rec[:, :, 0:H], fnq[:, :, CP:CP + H])   # xl
            nc.vector.tensor_copy(rec[:, :, H:H + C], pmat[:, :, 0:C])    # P
            nc.sync.dma_start(
                d_recout.ap().rearrange("(t p) c -> p t c", p=128), rec[:]
            )
            nc.sync.dma_start(d_xrout.ap(),
                              fnq[:, :, CP + H:CP + 2 * H])
            nc.sync.dma_start(d_lout.ap(), lmat[:])
            nc.sync.dma_start(d_pout.ap(), pmat[:])

    nc.compile()
    return nc


# --------------------------------------------------------------------------
# launch B: edge softmax + CE partial (node-major slot table)
# --------------------------------------------------------------------------
def build_program_b(D: int):
    bpk = B_SM + T * D
    nc = bacc.Bacc("TRN2", target_bir_lowering=False, debug=False,
                   num_devices=NCORES)

    d_recg = nc.dram_tensor("recg", [128, T * D * REC], BF16, kind="ExternalInput")
    d_pk = nc.dram_tensor("pk", [128, bpk], F32, kind="ExternalInput")
    d_scal = nc.dram_tensor("scal", [128, 1], F32, kind="ExternalOutput")

    with tile.TileContext(nc) as tc:
        with tc.tile_pool(name="sb", bufs=1) as sb:
            recg = sb.tile([128, T * D, REC], BF16)
            nc.sync.dma_start(recg[:], d_recg.ap())
            pk = sb.tile([128, bpk], F32)
            nc.sync.dma_start(pk[:], d_pk.ap())
            xr = pk[:, B_XR:B_XR + T * H].rearrange("p (t h) -> p t h", h=H)
            lmat = pk[:, B_LM:B_LM + T * CP].rearrange("p (t c) -> p t c", c=CP)
            attb = pk[:, B_AT:B_AT + H]
            smask = pk[:, B_SM:B_SM + T * D].rearrange("p (t d) -> p t d", d=D)

            recg4 = recg[:].rearrange("p (t d) r -> p t d r", d=D)
            xr_bc = xr.unsqueeze(2).broadcast_to([128, T, D, H])
            h = sb.tile([128, T, D, H], F32)
            nc.vector.tensor_tensor(h[:], recg4[:, :, :, 0:H], xr_bc, op=ALU.add)
            lr = sb.tile([128, T, D, H], F32)
            hs = sb.tile([128, T, D, H], F32)
            nc.scalar.mul(hs[:], h[:], NEG_SLOPE)
            nc.vector.tensor_tensor(lr[:], h[:], hs[:], op=ALU.max)
            ez = sb.tile([128, T, D, H], F32)
            att_bc = attb.unsqueeze(1).unsqueeze(1).broadcast_to([128, T, D, H])
            nc.vector.tensor_tensor(ez[:], lr[:], att_bc, op=ALU.mult)
            e = sb.tile([128, T, D], F32)
            nc.vector.tensor_reduce(e[:], ez[:], axis=AX.X, op=ALU.add)
            ex = sb.tile([128, T, D], F32)
            nc.scalar.activation(ex[:], e[:], AF.Exp)
            exm = sb.tile([128, T, D], F32)
            nc.vector.tensor_tensor(exm[:], ex[:], smask, op=ALU.mult)
            den = sb.tile([128, T], F32)
            nc.vector.tensor_reduce(den[:], exm[:], axis=AX.X, op=ALU.add)
            nc.vector.tensor_scalar(den[:], den[:], 1e-30, None, op0=ALU.add)
            rden = sb.tile([128, T], F32)
            nc.vector.reciprocal(rden[:], den[:])
            alpha = sb.tile([128, T, D], F32)
            nc.vector.tensor_tensor(
                alpha[:], exm[:], rden[:].unsqueeze(2).broadcast_to([128, T, D]),
                op=ALU.mult,
            )
            kz = sb.tile([128, T, D, C], F32)
            l_bc = lmat[:, :, 0:C].unsqueeze(2).broadcast_to([128, T, D, C])
            nc.vector.tensor_tensor(kz[:], recg4[:, :, :, H:H + C], l_bc,
                                    op=ALU.mult)
            kv = sb.tile([128, T, D], F32)
            nc.vector.tensor_reduce(kv[:], kz[:], axis=AX.X, op=ALU.add)
            ak = sb.tile([128, T, D], F32)
            nc.vector.tensor_tensor(ak[:], alpha[:], kv[:], op=ALU.mult)
            cev = sb.tile([128, 1], F32)
            nc.vector.tensor_reduce(cev[:], ak[:], axis=AX.XY, op=ALU.add)
            nc.sync.dma_start(d_scal.ap(), cev[:])

    nc.compile()
    return nc


# --------------------------------------------------------------------------
# host-side sharding
# --------------------------------------------------------------------------
def shard_inputs(x_sub, Mu, Var, edge_index_sub, subset_idx,
                 W_param, S_param, W_l, b_l, W_r, b_r, att):
    x_sub = np.asarray(x_sub, np.float32)
    Mu = np.asarray(Mu, np.float32)
    Var = np.asarray(Var, np.float32)
    ei = np.asarray(edge_index_sub).astype(np.int64)
    sub = np.asarray(subset_idx).astype(np.int64)
    W_param = np.asarray(W_param, np.float32)
    S_param = np.asarray(S_param, np.float32)
    W_l = np.asarray(W_l, np.float32)
    W_r = np.asarray(W_r, np.float32)
    b_l = np.asarray(b_l, np.float32)
    b_r = np.asarray(b_r, np.float32)
    att = np.asarray(att, np.float32)

    src, dst = ei[0], ei[1]
    deg = np.bincount(dst, minlength=N)
    D = max(int(deg.max()), 1)

    def rearr(a, w):
        # [GP, w] -> [128, 4*w] in (p, k, c) layout
        return a.reshape(4, 128, w).transpose(1, 0, 2).reshape(128, 4 * w)

    mut = np.zeros((GP, CP), np.float32)
    mut[:G, :C] = Mu.T
    vart = np.ones((GP, CP), np.float32)
    vart[:G, :C] = Var.T
    vart[G, :] = 1e30              # S-row of x^2: killed in A; d patched in
    vart[G + 1, :] = 1e30          # ones-row of x^2 must not contribute
    wl = np.zeros((GP, H), np.float32)
    wl[:G] = W_l
    wl[G + 1] = b_l                # bias via ones-row of x
    wr = np.zeros((GP, H), np.float32)
    wr[:G] = W_r
    wr[G + 1] = b_r
    ident = np.eye(128, dtype=np.float32)

    Wsub_all = W_param[sub]        # host row-gather, per sharding hint
    Ssub_all = S_param[sub, 0]

    in_maps_a = []
    aux = []
    for m in range(NCORES):
        lo, hi = m * NPC, (m + 1) * NPC
        sv = np.zeros(NM, np.float32)
        sv[:NPC] = Ssub_all[lo:hi]
        xtb = np.zeros((GP, NM), np.float32)
        xtb[:G, :NPC] = x_sub[lo:hi].T
        xtb[G, :] = sv                 # S row -> x^2 row 500 = S^2
        xtb[G + 1, :NPC] = 1.0         # ones row (biases)
        wn = np.full((NM, CP), WPAD, np.float32)
        wn[:NPC, :C] = Wsub_all[lo:hi]
        wn[NPC:, :] = 0.0
        wnode = wn.reshape(T, 128, CP).transpose(1, 0, 2).reshape(128, T * CP)
        scol = sv.reshape(T, 128).T
        pk = np.concatenate([
            wnode, scol, ident, rearr(mut, CP), rearr(vart, CP),
            rearr(wl, H), rearr(wr, H),
        ], axis=1).astype(np.float32)
        assert pk.shape[1] == APK
        # edges of this core, slotted by local destination
        sel = (dst >= lo) & (dst < hi)
        sm, dm = src[sel], dst[sel] - lo
        order = np.argsort(dm, kind="stable")
        ds, ss = dm[order], sm[order]
        seg_start = np.searchsorted(ds, ds)
        slot = np.arange(len(ds)) - seg_start
        slot_src = np.zeros((NM, D), np.int64)
        slot_mask = np.zeros((NM, D), np.float32)
        gid = (ss // NPC) * NM + (ss % NPC)   # padded global node id
        slot_src[ds, slot] = gid
        slot_mask[ds, slot] = 1.0
        sgid = slot_src.reshape(T, 128, D).transpose(1, 0, 2).reshape(128, T * D)
        smask = slot_mask.reshape(T, 128, D).transpose(1, 0, 2).reshape(128, T * D)
        in_maps_a.append({
            "xtb": xtb.astype(ml_dtypes.bfloat16),
            "pk": np.ascontiguousarray(pk),
        })
        aux.append({
            "sgid": sgid,
            "smask": np.ascontiguousarray(smask),
            "attb": np.tile(att[None, :], (128, 1)).astype(np.float32),
        })
    return in_maps_a, aux, D


def unshard_p(results_a):
    pparts = []
    for r in results_a:
        p = np.asarray(r["pout"]).reshape(128, T, CP).transpose(1, 0, 2)
        pparts.append(p.reshape(NM, CP)[:NPC, :C])
    return np.concatenate(pparts, axis=0).astype(np.float32)


# --------------------------------------------------------------------------
# entry point
# --------------------------------------------------------------------------
_last_res = None


def kernel(**inputs):
    global _last_res
    in_maps_a, aux, D = shard_inputs(**inputs)
    if "A" not in _prog_cache:
        _prog_cache["A"] = build_program_a()
    if ("B", D) not in _prog_cache:
        _prog_cache[("B", D)] = build_program_b(D)

    res_a = run_bass_kernel_spmd(_prog_cache["A"], in_maps_a,
                                 list(range(NCORES)))
    # exchange: concatenate record tables, gather per-edge-slot records
    rec_all = np.concatenate(
        [np.asarray(r["recout"], np.float32) for r in res_a.results], axis=0
    )
    in_maps_b = []
    for m in range(NCORES):
        recg = rec_all[aux[m]["sgid"].reshape(-1)].reshape(128, T * D * REC)
        pk = np.concatenate([
            np.asarray(res_a.results[m]["xrout"], np.float32),
            np.asarray(res_a.results[m]["lout"], np.float32),
            aux[m]["attb"],
            aux[m]["smask"],
        ], axis=1).astype(np.float32)
        in_maps_b.append({
            "recg": recg.astype(ml_dtypes.bfloat16),
            "pk": np.ascontiguousarray(pk),
        })
    res_b = run_bass_kernel_spmd(_prog_cache[("B", D)], in_maps_b,
                                 list(range(NCORES)))
    _last_res = (res_a, res_b)

    ll = sum(float(np.asarray(r["scal"], np.float64).sum()) for r in res_a.results)
    ce = sum(float(np.asarray(r["scal"], np.float64).sum()) for r in res_b.results)
    P = unshard_p(res_a.results)
    return np.float32(ll / N), np.float32(-ce / N), P


if __name__ == "__main__":
    import reference

    inp = reference.setup_inputs()
    inp = {k: np.asarray(v) for k, v in inp.items()}
    out = kernel(**inp)
    print("ll", out[0], "ce", out[1], "P", out[2].shape)


# revision 14
# speedup vs baseline: 1.1133x; 1.0906x over previous
"""Trainium2 Bass kernel for nn_Batched_STHD_SpGAT (gnn_message_passing).

Computes, on 8 NeuronCores (SPMD, node-sharded):
  ll_prot = sum(P_sub * F_c)/n           (Gaussian log-likelihood term)
  ce_space = -sum(P[src]*alpha*log(P[dst]+eps))/n   (GATv2 edge-softmax CE)
  P_sub = softmax(W_param[subset_idx], axis=1)

Sharding: nodes (and their incoming edges) are partitioned across 8 cores.
Launch A computes, per core, the Gaussian term F, P = softmax(W), the GAT
projections xl/xr and the ll partial, and emits a per-node record table
[xl | P].  The record tables are exchanged/gathered into per-edge-slot
records (destination-ordered slot table, all cores' records concatenated),
and launch B computes the edge softmax + CE partials on-device.

Algebra on device:
  F[n,c] = -0.5*sum_g (x-Mu*S)^2/Var
         = sum_g x^2 * (-0.5/Var)  + S * sum_g x*Mu/Var  - 0.5*S^2*sum_g Mu^2/Var
  Gene-row 500 of x holds S (so x^2 row 500 is S^2 automatically) and the A
  matrix row 500 is patched to -0.5*d (d = sum_g Mu^2/Var), folding the
  -0.5*S^2*d term into the x^2 matmul; the b_l/b_r biases are folded into
  the xl/xr matmul via a ones row of x (row 501).
  Edge softmax is computed without the segment-max shift (scores are O(1)).
"""

import sys

sys.path.insert(0, "/opt/trn_rl_repo")

import numpy as np
import ml_dtypes

import concourse.bacc as bacc
import concourse.tile as tile
from concourse import mybir
from concourse.bass_utils import run_bass_kernel_spmd

F32 = mybir.dt.float32
BF16 = mybir.dt.bfloat16
AF = mybir.ActivationFunctionType
ALU = mybir.AluOpType
AX = mybir.AxisListType

# problem constants (hardcoded per contest contract)
NCORES = 8
N = 10000           # nodes total
NPC = N // NCORES   # real nodes per core (1250)
T = 10              # node tiles per core
NM = 128 * T        # padded nodes per core (1280)
NMG = NM * NCORES   # padded global nodes (10240)
G = 500             # genes
GP = 512            # padded genes (4 chunks of 128)
C = 17              # classes
CP = 32             # padded classes
H = 8               # GAT hidden
REC = 26            # record row width: [xl(8) | P(17) | pad]
NEG_SLOPE = 0.2
WPAD = -100.0       # pad-class logit

# packed small-input column offsets for launch A ([128, APK] f32):
#  wnode [T*CP] | scol [T] | ident [128] | mut [4*CP] | vart [4*CP]
#  | wl [4*H] | wr [4*H]
A_WN, A_SC, A_ID = 0, T * CP, T * CP + T
A_MU = A_ID + 128
A_VA = A_MU + 4 * CP
A_WL = A_VA + 4 * CP
A_WR = A_WL + 4 * H
APK = A_WR + 4 * H
# packed small-input column offsets for launch B ([128, BPK(D)] f32):
#  xr [T*H] | lmat [T*CP] | attb [H] | smask [T*D]
B_XR, B_LM, B_AT, B_SM = 0, T * H, T * H + T * CP, T * H + T * CP + H

_prog_cache = {}


# --------------------------------------------------------------------------
# launch A: Gaussian term, softmax, GAT projections, ll partial, records
# --------------------------------------------------------------------------
def build_program_a():
    nc = bacc.Bacc("TRN2", target_bir_lowering=False, debug=False,
                   num_devices=NCORES)

    d_xtb = nc.dram_tensor("xtb", [GP, NM], BF16, kind="ExternalInput")
    d_pk = nc.dram_tensor("pk", [128, APK], F32, kind="ExternalInput")

    d_pout = nc.dram_tensor("pout", [128, T * CP], F32, kind="ExternalOutput")
    d_lout = nc.dram_tensor("lout", [128, T * CP], F32, kind="ExternalOutput")
    d_xrout = nc.dram_tensor("xrout", [128, T * H], F32, kind="ExternalOutput")
    d_recout = nc.dram_tensor("recout", [NM, REC], F32, kind="ExternalOutput")
    d_scal = nc.dram_tensor("scal", [128, 1], F32, kind="ExternalOutput")

    with tile.TileContext(nc) as tc:
        with (
            tc.tile_pool(name="sb", bufs=1) as sb,
            tc.tile_pool(name="dramp", bufs=1, space="DRAM") as dpool,
        ):
            # ======== loads ========
            xtb = sb.tile([128, 4, NM], BF16)
            nc.sync.dma_start(xtb[:], d_xtb.ap().rearrange("(k p) n -> p k n", p=128))
            pk = sb.tile([128, APK], F32)
            nc.sync.dma_start(pk[:], d_pk.ap())
            wnode = pk[:, A_WN:A_WN + T * CP].rearrange("p (t c) -> p t c", c=CP)
            scol = pk[:, A_SC:A_SC + T]
            ident = pk[:, A_ID:A_ID + 128]
            mut = pk[:, A_MU:A_MU + 4 * CP].rearrange("p (k c) -> p k c", c=CP)
            vart = pk[:, A_VA:A_VA + 4 * CP].rearrange("p (k c) -> p k c", c=CP)
            wlt = pk[:, A_WL:A_WL + 4 * H].rearrange("p (k h) -> p k h", h=H)
            wrt = pk[:, A_WR:A_WR + 4 * H].rearrange("p (k h) -> p k h", h=H)

            # ======== weight prep ========
            recipv = sb.tile([128, 4, CP], F32)
            nc.vector.reciprocal(recipv[:], vart)
            btf = sb.tile([128, 4, CP], F32)          # B = Mu/Var (f32)
            nc.vector.tensor_tensor(btf[:], mut, recipv[:], op=ALU.mult)
            wcat = sb.tile([128, 4, CP + 2 * H], BF16)
            nc.vector.tensor_copy(wcat[:, :, 0:CP], btf[:])
            nc.vector.tensor_copy(wcat[:, :, CP:CP + H], wlt)
            nc.vector.tensor_copy(wcat[:, :, CP + H:CP + 2 * H], wrt)
            acat = sb.tile([128, 4, CP], BF16)        # -0.5/Var
            nc.scalar.mul(acat[:], recipv[:], -0.5)
            m2v = sb.tile([128, 4, CP], F32)
            nc.vector.tensor_tensor(m2v[:], mut, btf[:], op=ALU.mult)
            ones = sb.tile([128, 1], F32)
            nc.vector.memset(ones[:], 1.0)
            with tc.tile_pool(name="psd", bufs=1, space="PSUM") as psd:
                dps = psd.tile([1, CP], F32)
                for k in range(4):
                    nc.tensor.matmul(
                        dps[:], ones[:], m2v[:, k, :], start=(k == 0), stop=(k == 3)
                    )
                drow = sb.tile([1, CP], BF16)
                nc.scalar.mul(drow[:], dps[:], -0.5)   # -0.5*d, cast bf16
            # patch Acat gene-row 500 (chunk 3, partition 116) via DRAM bounce
            drow_d = dpool.tile([1, CP], BF16)
            nc.sync.dma_start(drow_d[:], drow[:])
            nc.sync.dma_start(acat[116:117, 3, :], drow_d[:])

            # x^2 (bf16); x row 500 = S (host), so x^2 row 500 = S^2
            x2b = sb.tile([128, 4, NM], BF16)
            nc.vector.tensor_tensor(
                x2b[:, 0:3, :], xtb[:, 0:3, :], xtb[:, 0:3, :], op=ALU.mult
            )
            nc.scalar.activation(x2b[:, 3, :], xtb[:, 3, :], AF.Square)

            # ======== main matmuls (class-major, stationary weights) ========
            f1 = sb.tile([CP + 2 * H, NM], F32)
            q = sb.tile([CP, NM], F32)
            with tc.tile_pool(name="psm", bufs=1, space="PSUM") as psm:
                o1ps = psm.tile([CP + 2 * H, NM], F32)
                qps = psm.tile([CP, NM], F32)
                for b, w in [(0, 512), (512, 512), (1024, 256)]:
                    for k in range(4):
                        nc.tensor.matmul(
                            o1ps[:, b:b + w], wcat[:, k, :], xtb[:, k, b:b + w],
                            start=(k == 0), stop=(k == 3),
                        )
                    for k in range(4):
                        nc.tensor.matmul(
                            qps[:, b:b + w], acat[:, k, :], x2b[:, k, b:b + w],
                            start=(k == 0), stop=(k == 3),
                        )
                nc.vector.tensor_copy(f1[:], o1ps[:])
                nc.scalar.copy(q[:], qps[:])

            # ======== transpose to node-major ========
            fnq = sb.tile([128, T, 80], F32)
            with tc.tile_pool(name="pst", bufs=1, space="PSUM") as pst:
                tra = pst.tile([128, 6 * 80], F32, name="tra")
                trb = pst.tile([128, 4 * 80], F32, name="trb")
                for t in range(T):
                    dst = tra if t < 6 else trb
                    off = (t if t < 6 else t - 6) * 80
                    nc.tensor.transpose(
                        dst[:, off:off + 48], f1[:, 128 * t:128 * (t + 1)],
                        ident[0:CP + 2 * H, 0:CP + 2 * H],
                    )
                    nc.tensor.transpose(
                        dst[:, off + 48:off + 80], q[:, 128 * t:128 * (t + 1)],
                        ident[0:CP, 0:CP],
                    )
                nc.vector.tensor_copy(fnq[:, 0:6, :], tra[:])
                nc.vector.tensor_copy(fnq[:, 6:10, :], trb[:])

            # ======== F (node-major) ========
            scol_bc = scol.unsqueeze(2).broadcast_to([128, T, CP])
            fmat = sb.tile([128, T, CP], F32)
            nc.vector.tensor_tensor(fmat[:], fnq[:, :, 0:CP], scol_bc, op=ALU.mult)
            nc.vector.tensor_tensor(fmat[:], fmat[:], fnq[:, :, 48:80], op=ALU.add)

            # ======== P softmax (node-major, |W| < 1 so no max shift) ========
            expw = sb.tile([128, T, CP], F32)
            nc.scalar.activation(expw[:], wnode, AF.Exp)
            rs = sb.tile([128, T], F32)
            nc.vector.tensor_reduce(rs[:], expw[:], axis=AX.X, op=ALU.add)
            rr = sb.tile([128, T], F32)
            nc.vector.reciprocal(rr[:], rs[:])
            pmat = sb.tile([128, T, CP], F32)
            nc.vector.tensor_tensor(
                pmat[:], expw[:], rr[:].unsqueeze(2).broadcast_to([128, T, CP]),
                op=ALU.mult,
            )
            lmat = sb.tile([128, T, CP], F32)
            peps = sb.tile([128, T, CP], F32)
            nc.vector.tensor_scalar(peps[:], pmat[:], 1e-8, None, op0=ALU.add)
            nc.scalar.activation(lmat[:], peps[:], AF.Ln)

            # ll partial (per-partition; host sums the 128 values)
            pf = sb.tile([128, T, CP], F32)
            nc.vector.tensor_tensor(pf[:], pmat[:], fmat[:], op=ALU.mult)
            llv = sb.tile([128, 1], F32)
            nc.vector.tensor_reduce(llv[:], pf[:], axis=AX.XY, op=ALU.add)
            nc.sync.dma_start(d_scal.ap(), llv[:])

            # ======== per-node record table [xl | P] ========
            rec = sb.tile([128, T, REC], F32)
            nc.vector.memset(rec[:, :, H + C:REC], 0.0)
            nc.vector.tensor_copy(rec[:, :, 0:H], fnq[:, :, CP:CP + H])   # xl
            nc.vector.tensor_copy(rec[:, :, H:H + C], pmat[:, :, 0:C])    # P
            nc.sync.dma_start(
                d_recout.ap().rearrange("(t p) c -> p t c", p=128), rec[:]
            )
            nc.sync.dma_start(d_xrout.ap(),
                              fnq[:, :, CP + H:CP + 2 * H])
            nc.sync.dma_start(d_lout.ap(), lmat[:])
            nc.sync.dma_start(d_pout.ap(), pmat[:])

    nc.compile()
    return nc


# --------------------------------------------------------------------------
# launch B: edge softmax + CE partial (node-major slot table)
# --------------------------------------------------------------------------
def build_program_b(D: int):
    bpk = B_SM + T * D
    nc = bacc.Bacc("TRN2", target_bir_lowering=False, debug=False,
                   num_devices=NCORES)

    d_recg = nc.dram_tensor("recg", [128, T * D * REC], BF16, kind="ExternalInput")
    d_pk = nc.dram_tensor("pk", [128, bpk], BF16, kind="ExternalInput")
    d_scal = nc.dram_tensor("scal", [128, 1], F32, kind="ExternalOutput")

    with tile.TileContext(nc) as tc:
        with tc.tile_pool(name="sb", bufs=1) as sb:
            recg = sb.tile([128, T * D, REC], BF16)
            nc.sync.dma_start(recg[:], d_recg.ap())
            pk = sb.tile([128, bpk], BF16)
            nc.sync.dma_start(pk[:], d_pk.ap())
            xr = pk[:, B_XR:B_XR + T * H].rearrange("p (t h) -> p t h", h=H)
            lmat = pk[:, B_LM:B_LM + T * CP].rearrange("p (t c) -> p t c", c=CP)
            attb = pk[:, B_AT:B_AT + H]
            smask = pk[:, B_SM:B_SM + T * D].rearrange("p (t d) -> p t d", d=D)

            recg4 = recg[:].rearrange("p (t d) r -> p t d r", d=D)
            xr_bc = xr.unsqueeze(2).broadcast_to([128, T, D, H])
            # edge-score chain in bf16 (DVE 2x/4x perf modes)
            h = sb.tile([128, T, D, H], BF16)
            nc.vector.tensor_tensor(h[:], recg4[:, :, :, 0:H], xr_bc, op=ALU.add)
            lr = sb.tile([128, T, D, H], BF16)
            hs = sb.tile([128, T, D, H], BF16)
            nc.scalar.mul(hs[:], h[:], NEG_SLOPE)
            nc.vector.tensor_tensor(lr[:], h[:], hs[:], op=ALU.max)
            ez = sb.tile([128, T, D, H], BF16)
            att_bc = attb.unsqueeze(1).unsqueeze(1).broadcast_to([128, T, D, H])
            nc.vector.tensor_tensor(ez[:], lr[:], att_bc, op=ALU.mult)
            e = sb.tile([128, T, D], F32)
            nc.vector.tensor_reduce(e[:], ez[:], axis=AX.X, op=ALU.add)
            ex = sb.tile([128, T, D], F32)
            nc.scalar.activation(ex[:], e[:], AF.Exp)
            exm = sb.tile([128, T, D], F32)
            nc.vector.tensor_tensor(exm[:], ex[:], smask, op=ALU.mult)
            den = sb.tile([128, T], F32)
            nc.vector.tensor_reduce(den[:], exm[:], axis=AX.X, op=ALU.add)
            nc.vector.tensor_scalar(den[:], den[:], 1e-30, None, op0=ALU.add)
            rden = sb.tile([128, T], F32)
            nc.vector.reciprocal(rden[:], den[:])
            alpha = sb.tile([128, T, D], F32)
            nc.vector.tensor_tensor(
                alpha[:], exm[:], rden[:].unsqueeze(2).broadcast_to([128, T, D]),
                op=ALU.mult,
            )
            kz = sb.tile([128, T, D, C], BF16)
            l_bc = lmat[:, :, 0:C].unsqueeze(2).broadcast_to([128, T, D, C])
            nc.vector.tensor_tensor(kz[:], recg4[:, :, :, H:H + C], l_bc,
                                    op=ALU.mult)
            kv = sb.tile([128, T, D], F32)
            nc.vector.tensor_reduce(kv[:], kz[:], axis=AX.X, op=ALU.add)
            ak = sb.tile([128, T, D], F32)
            nc.vector.tensor_tensor(ak[:], alpha[:], kv[:], op=ALU.mult)
            cev = sb.tile([128, 1], F32)
            nc.vector.tensor_reduce(cev[:], ak[:], axis=AX.XY, op=ALU.add)
            nc.sync.dma_start(d_scal.ap(), cev[:])

    nc.compile()
    return nc


# --------------------------------------------------------------------------
# host-side sharding
# --------------------------------------------------------------------------
def shard_inputs(x_sub, Mu, Var, edge_index_sub, subset_idx,
                 W_param, S_param, W_l, b_l, W_r, b_r, att):
    x_sub = np.asarray(x_sub, np.float32)
    Mu = np.asarray(Mu, np.float32)
    Var = np.asarray(Var, np.float32)
    ei = np.asarray(edge_index_sub).astype(np.int64)
    sub = np.asarray(subset_idx).astype(np.int64)
    W_param = np.asarray(W_param, np.float32)
    S_param = np.asarray(S_param, np.float32)
    W_l = np.asarray(W_l, np.float32)
    W_r = np.asarray(W_r, np.float32)
    b_l = np.asarray(b_l, np.float32)
    b_r = np.asarray(b_r, np.float32)
    att = np.asarray(att, np.float32)

    src, dst = ei[0], ei[1]
    deg = np.bincount(dst, minlength=N)
    D = max(int(deg.max()), 1)

    def rearr(a, w):
        # [GP, w] -> [128, 4*w] in (p, k, c) layout
        return a.reshape(4, 128, w).transpose(1, 0, 2).reshape(128, 4 * w)

    mut = np.zeros((GP, CP), np.float32)
    mut[:G, :C] = Mu.T
    vart = np.ones((GP, CP), np.float32)
    vart[:G, :C] = Var.T
    vart[G, :] = 1e30              # S-row of x^2: killed in A; d patched in
    vart[G + 1, :] = 1e30          # ones-row of x^2 must not contribute
    wl = np.zeros((GP, H), np.float32)
    wl[:G] = W_l
    wl[G + 1] = b_l                # bias via ones-row of x
    wr = np.zeros((GP, H), np.float32)
    wr[:G] = W_r
    wr[G + 1] = b_r
    ident = np.eye(128, dtype=np.float32)

    Wsub_all = W_param[sub]        # host row-gather, per sharding hint
    Ssub_all = S_param[sub, 0]

    in_maps_a = []
    aux = []
    for m in range(NCORES):
        lo, hi = m * NPC, (m + 1) * NPC
        sv = np.zeros(NM, np.float32)
        sv[:NPC] = Ssub_all[lo:hi]
        xtb = np.zeros((GP, NM), np.float32)
        xtb[:G, :NPC] = x_sub[lo:hi].T
        xtb[G, :] = sv                 # S row -> x^2 row 500 = S^2
        xtb[G + 1, :NPC] = 1.0         # ones row (biases)
        wn = np.full((NM, CP), WPAD, np.float32)
        wn[:NPC, :C] = Wsub_all[lo:hi]
        wn[NPC:, :] = 0.0
        wnode = wn.reshape(T, 128, CP).transpose(1, 0, 2).reshape(128, T * CP)
        scol = sv.reshape(T, 128).T
        pk = np.concatenate([
            wnode, scol, ident, rearr(mut, CP), rearr(vart, CP),
            rearr(wl, H), rearr(wr, H),
        ], axis=1).astype(np.float32)
        assert pk.shape[1] == APK
        # edges of this core, slotted by local destination
        sel = (dst >= lo) & (dst < hi)
        sm, dm = src[sel], dst[sel] - lo
        order = np.argsort(dm, kind="stable")
        ds, ss = dm[order], sm[order]
        seg_start = np.searchsorted(ds, ds)
        slot = np.arange(len(ds)) - seg_start
        slot_src = np.zeros((NM, D), np.int64)
        slot_mask = np.zeros((NM, D), np.float32)
        gid = (ss // NPC) * NM + (ss % NPC)   # padded global node id
        slot_src[ds, slot] = gid
        slot_mask[ds, slot] = 1.0
        sgid = slot_src.reshape(T, 128, D).transpose(1, 0, 2).reshape(128, T * D)
        smask = slot_mask.reshape(T, 128, D).transpose(1, 0, 2).reshape(128, T * D)
        in_maps_a.append({
            "xtb": xtb.astype(ml_dtypes.bfloat16),
            "pk": np.ascontiguousarray(pk),
        })
        aux.append({
            "sgid": sgid,
            "smask": np.ascontiguousarray(smask),
            "attb": np.tile(att[None, :], (128, 1)).astype(np.float32),
        })
    return in_maps_a, aux, D


def unshard_p(results_a):
    pparts = []
    for r in results_a:
        p = np.asarray(r["pout"]).reshape(128, T, CP).transpose(1, 0, 2)
        pparts.append(p.reshape(NM, CP)[:NPC, :C])
    return np.concatenate(pparts, axis=0).astype(np.float32)


# --------------------------------------------------------------------------
# entry point
# --------------------------------------------------------------------------
_last_res = None


def kernel(**inputs):
    global _last_res
    in_maps_a, aux, D = shard_inputs(**inputs)
    if "A" not in _prog_cache:
        _prog_cache["A"] = build_program_a()
    if ("B", D) not in _prog_cache:
        _prog_cache[("B", D)] = build_program_b(D)

    res_a = run_bass_kernel_spmd(_prog_cache["A"], in_maps_a,
                                 list(range(NCORES)))
    # exchange: concatenate record tables, gather per-edge-slot records
    rec_all = np.concatenate(
        [np.asarray(r["recout"], np.float32) for r in res_a.results], axis=0
    )
    in_maps_b = []
    for m in range(NCORES):
        recg = rec_all[aux[m]["sgid"].reshape(-1)].reshape(128, T * D * REC)
        pk = np.concatenate([
            np.asarray(res_a.results[m]["xrout"], np.float32),
            np.asarray(res_a.results[m]["lout"], np.float32),
            aux[m]["attb"],
            aux[m]["smask"],
        ], axis=1).astype(np.float32)
        in_maps_b.append({
            "recg": recg.astype(ml_dtypes.bfloat16),
            "pk": np.ascontiguousarray(pk).astype(ml_dtypes.bfloat16),
        })
    res_b = run_bass_kernel_spmd(_prog_cache[("B", D)], in_maps_b,
                                 list(range(NCORES)))
    _last_res = (res_a, res_b)

    ll = sum(float(np.asarray(r["scal"], np.float64).sum()) for r in res_a.results)
    ce = sum(float(np.asarray(r["scal"], np.float64).sum()) for r in res_b.results)
    P = unshard_p(res_a.results)
    return np.float32(ll / N), np.float32(-ce / N), P


if __name__ == "__main__":
    import reference

    inp = reference.setup_inputs()
    inp = {k: np.asarray(v) for k, v in inp.items()}
    out = kernel(**inp)
    print("ll", out[0], "ce", out[1], "P", out[2].shape)


# revision 15
# speedup vs baseline: 1.1692x; 1.0502x over previous
"""Trainium2 Bass kernel for nn_Batched_STHD_SpGAT (gnn_message_passing).

Computes, on 8 NeuronCores (SPMD, node-sharded):
  ll_prot = sum(P_sub * F_c)/n           (Gaussian log-likelihood term)
  ce_space = -sum(P[src]*alpha*log(P[dst]+eps))/n   (GATv2 edge-softmax CE)
  P_sub = softmax(W_param[subset_idx], axis=1)

Sharding: nodes (and their incoming edges) are partitioned across 8 cores.
Launch A computes, per core, the Gaussian term F, P = softmax(W), the GAT
projections xl/xr and the ll partial, and emits a per-node record table
[xl | P].  The record tables are exchanged/gathered into per-edge-slot
records (destination-ordered slot table, all cores' records concatenated),
and launch B computes the edge softmax + CE partials on-device.

Algebra on device:
  F[n,c] = -0.5*sum_g (x-Mu*S)^2/Var
         = sum_g x^2 * (-0.5/Var)  + S * sum_g x*Mu/Var  - 0.5*S^2*sum_g Mu^2/Var
  Gene-row 500 of x holds S (so x^2 row 500 is S^2 automatically) and the A
  matrix row 500 is patched to -0.5*d (d = sum_g Mu^2/Var), folding the
  -0.5*S^2*d term into the x^2 matmul; the b_l/b_r biases are folded into
  the xl/xr matmul via a ones row of x (row 501).
  Edge softmax is computed without the segment-max shift (scores are O(1)).
"""

import sys

sys.path.insert(0, "/opt/trn_rl_repo")

import numpy as np
import ml_dtypes

import concourse.bacc as bacc
import concourse.tile as tile
from concourse import mybir
from concourse.bass_utils import run_bass_kernel_spmd

F32 = mybir.dt.float32
BF16 = mybir.dt.bfloat16
AF = mybir.ActivationFunctionType
ALU = mybir.AluOpType
AX = mybir.AxisListType

# problem constants (hardcoded per contest contract)
NCORES = 8
N = 10000           # nodes total
NPC = N // NCORES   # real nodes per core (1250)
T = 10              # node tiles per core
NM = 128 * T        # padded nodes per core (1280)
NMG = NM * NCORES   # padded global nodes (10240)
G = 500             # genes
GP = 512            # padded genes (4 chunks of 128)
C = 17              # classes
CP = 32             # padded classes
H = 8               # GAT hidden
REC = 26            # record row width: [xl(8) | P(17) | pad]
NEG_SLOPE = 0.2
WPAD = -100.0       # pad-class logit

# packed small-input column offsets for launch A ([128, APK] f32):
#  wnode [T*CP] | scol [T] | ident [128] | mut [4*CP] | vart [4*CP]
#  | wl [4*H] | wr [4*H]
A_WN, A_SC, A_ID = 0, T * CP, T * CP + T
A_MU = A_ID + 128
A_VA = A_MU + 4 * CP
A_WL = A_VA + 4 * CP
A_WR = A_WL + 4 * H
APK = A_WR + 4 * H
# packed small-input column offsets for launch B ([128, BPK(D)] f32):
#  xr [T*H] | lmat [T*CP] | attb [H] | smask [T*D]
B_XR, B_LM, B_AT, B_SM = 0, T * H, T * H + T * CP, T * H + T * CP + H

_prog_cache = {}


# --------------------------------------------------------------------------
# launch A: Gaussian term, softmax, GAT projections, ll partial, records
# --------------------------------------------------------------------------
def build_program_a():
    nc = bacc.Bacc("TRN2", target_bir_lowering=False, debug=False,
                   num_devices=NCORES)

    d_xtb = nc.dram_tensor("xtb", [GP, NM], BF16, kind="ExternalInput")
    d_pk = nc.dram_tensor("pk", [128, APK], F32, kind="ExternalInput")

    d_pout = nc.dram_tensor("pout", [128, T * CP], F32, kind="ExternalOutput")
    d_lout = nc.dram_tensor("lout", [128, T * CP], F32, kind="ExternalOutput")
    d_xrout = nc.dram_tensor("xrout", [128, T * H], F32, kind="ExternalOutput")
    d_recout = nc.dram_tensor("recout", [NM, REC], F32, kind="ExternalOutput")
    d_scal = nc.dram_tensor("scal", [128, 1], F32, kind="ExternalOutput")

    with tile.TileContext(nc) as tc:
        with (
            tc.tile_pool(name="sb", bufs=1) as sb,
            tc.tile_pool(name="dramp", bufs=1, space="DRAM") as dpool,
        ):
            # ======== loads ========
            xtb = sb.tile([128, 4, NM], BF16)
            xsrc = d_xtb.ap().rearrange("(k p) n -> p k n", p=128)
            for k in range(4):
                nc.sync.dma_start(xtb[:, k, :], xsrc[:, k, :])
            pk = sb.tile([128, APK], F32)
            nc.sync.dma_start(pk[:], d_pk.ap())
            wnode = pk[:, A_WN:A_WN + T * CP].rearrange("p (t c) -> p t c", c=CP)
            scol = pk[:, A_SC:A_SC + T]
            ident = pk[:, A_ID:A_ID + 128]
            mut = pk[:, A_MU:A_MU + 4 * CP].rearrange("p (k c) -> p k c", c=CP)
            vart = pk[:, A_VA:A_VA + 4 * CP].rearrange("p (k c) -> p k c", c=CP)
            wlt = pk[:, A_WL:A_WL + 4 * H].rearrange("p (k h) -> p k h", h=H)
            wrt = pk[:, A_WR:A_WR + 4 * H].rearrange("p (k h) -> p k h", h=H)

            # ======== weight prep ========
            recipv = sb.tile([128, 4, CP], F32)
            nc.vector.reciprocal(recipv[:], vart)
            btf = sb.tile([128, 4, CP], F32)          # B = Mu/Var (f32)
            nc.vector.tensor_tensor(btf[:], mut, recipv[:], op=ALU.mult)
            wcat = sb.tile([128, 4, CP + 2 * H], BF16)
            nc.vector.tensor_copy(wcat[:, :, 0:CP], btf[:])
            nc.vector.tensor_copy(wcat[:, :, CP:CP + H], wlt)
            nc.vector.tensor_copy(wcat[:, :, CP + H:CP + 2 * H], wrt)
            acat = sb.tile([128, 4, CP], BF16)        # -0.5/Var
            nc.scalar.mul(acat[:], recipv[:], -0.5)
            m2v = sb.tile([128, 4, CP], F32)
            nc.vector.tensor_tensor(m2v[:], mut, btf[:], op=ALU.mult)
            ones = sb.tile([128, 1], F32)
            nc.vector.memset(ones[:], 1.0)
            with tc.tile_pool(name="psd", bufs=1, space="PSUM") as psd:
                dps = psd.tile([1, CP], F32)
                for k in range(4):
                    nc.tensor.matmul(
                        dps[:], ones[:], m2v[:, k, :], start=(k == 0), stop=(k == 3)
                    )
                drow = sb.tile([1, CP], BF16)
                nc.scalar.mul(drow[:], dps[:], -0.5)   # -0.5*d, cast bf16
            # patch Acat gene-row 500 (chunk 3, partition 116) via DRAM bounce
            drow_d = dpool.tile([1, CP], BF16)
            nc.sync.dma_start(drow_d[:], drow[:])
            nc.sync.dma_start(acat[116:117, 3, :], drow_d[:])

            # x^2 (bf16); x row 500 = S (host), so x^2 row 500 = S^2
            x2b = sb.tile([128, 4, NM], BF16)
            for k in range(3):
                nc.vector.tensor_tensor(
                    x2b[:, k, :], xtb[:, k, :], xtb[:, k, :], op=ALU.mult
                )
            nc.scalar.activation(x2b[:, 3, :], xtb[:, 3, :], AF.Square)

            # ======== main matmuls (class-major, stationary weights) ========
            f1 = sb.tile([CP + 2 * H, NM], F32)
            q = sb.tile([CP, NM], F32)
            with tc.tile_pool(name="psm", bufs=1, space="PSUM") as psm:
                o1ps = psm.tile([CP + 2 * H, NM], F32)
                qps = psm.tile([CP, NM], F32)
                for b, w in [(0, 512), (512, 512), (1024, 256)]:
                    for k in range(4):
                        nc.tensor.matmul(
                            o1ps[:, b:b + w], wcat[:, k, :], xtb[:, k, b:b + w],
                            start=(k == 0), stop=(k == 3),
                        )
                    for k in range(4):
                        nc.tensor.matmul(
                            qps[:, b:b + w], acat[:, k, :], x2b[:, k, b:b + w],
                            start=(k == 0), stop=(k == 3),
                        )
                nc.vector.tensor_copy(f1[:], o1ps[:])
                nc.scalar.copy(q[:], qps[:])

            # ======== transpose to node-major ========
            fnq = sb.tile([128, T, 80], F32)
            with tc.tile_pool(name="pst", bufs=1, space="PSUM") as pst:
                tra = pst.tile([128, 6 * 80], F32, name="tra")
                trb = pst.tile([128, 4 * 80], F32, name="trb")
                for t in range(T):
                    dst = tra if t < 6 else trb
                    off = (t if t < 6 else t - 6) * 80
                    nc.tensor.transpose(
                        dst[:, off:off + 48], f1[:, 128 * t:128 * (t + 1)],
                        ident[0:CP + 2 * H, 0:CP + 2 * H],
                    )
                    nc.tensor.transpose(
                        dst[:, off + 48:off + 80], q[:, 128 * t:128 * (t + 1)],
                        ident[0:CP, 0:CP],
                    )
                nc.vector.tensor_copy(fnq[:, 0:6, :], tra[:])
                nc.vector.tensor_copy(fnq[:, 6:10, :], trb[:])

            # ======== F (node-major) ========
            scol_bc = scol.unsqueeze(2).broadcast_to([128, T, CP])
            fmat = sb.tile([128, T, CP], F32)
            nc.vector.tensor_tensor(fmat[:], fnq[:, :, 0:CP], scol_bc, op=ALU.mult)
            nc.vector.tensor_tensor(fmat[:], fmat[:], fnq[:, :, 48:80], op=ALU.add)

            # ======== P softmax (node-major, |W| < 1 so no max shift) ========
            expw = sb.tile([128, T, CP], F32)
            nc.scalar.activation(expw[:], wnode, AF.Exp)
            rs = sb.tile([128, T], F32)
            nc.vector.tensor_reduce(rs[:], expw[:], axis=AX.X, op=ALU.add)
            rr = sb.tile([128, T], F32)
            nc.vector.reciprocal(rr[:], rs[:])
            pmat = sb.tile([128, T, CP], F32)
            nc.vector.tensor_tensor(
                pmat[:], expw[:], rr[:].unsqueeze(2).broadcast_to([128, T, CP]),
                op=ALU.mult,
            )
            lmat = sb.tile([128, T, CP], F32)
            peps = sb.tile([128, T, CP], F32)
            nc.vector.tensor_scalar(peps[:], pmat[:], 1e-8, None, op0=ALU.add)
            nc.scalar.activation(lmat[:], peps[:], AF.Ln)

            # ll partial (per-partition; host sums the 128 values)
            pf = sb.tile([128, T, CP], F32)
            nc.vector.tensor_tensor(pf[:], pmat[:], fmat[:], op=ALU.mult)
            llv = sb.tile([128, 1], F32)
            nc.vector.tensor_reduce(llv[:], pf[:], axis=AX.XY, op=ALU.add)
            nc.sync.dma_start(d_scal.ap(), llv[:])

            # ======== per-node record table [xl | P] ========
            rec = sb.tile([128, T, REC], F32)
            nc.vector.memset(rec[:, :, H + C:REC], 0.0)
            nc.vector.tensor_copy(rec[:, :, 0:H], fnq[:, :, CP:CP + H])   # xl
            nc.vector.tensor_copy(rec[:, :, H:H + C], pmat[:, :, 0:C])    # P
            nc.sync.dma_start(
                d_recout.ap().rearrange("(t p) c -> p t c", p=128), rec[:]
            )
            nc.sync.dma_start(d_xrout.ap(),
                              fnq[:, :, CP + H:CP + 2 * H])
            nc.sync.dma_start(d_lout.ap(), lmat[:])
            nc.sync.dma_start(d_pout.ap(), pmat[:])

    nc.compile()
    return nc


# --------------------------------------------------------------------------
# launch B: edge softmax + CE partial (node-major slot table)
# --------------------------------------------------------------------------
def build_program_b(D: int):
    bpk = B_SM + T * D
    nc = bacc.Bacc("TRN2", target_bir_lowering=False, debug=False,
                   num_devices=NCORES)

    d_recg = nc.dram_tensor("recg", [128, T * D * REC], BF16, kind="ExternalInput")
    d_pk = nc.dram_tensor("pk", [128, bpk], BF16, kind="ExternalInput")
    d_scal = nc.dram_tensor("scal", [128, 1], F32, kind="ExternalOutput")

    with tile.TileContext(nc) as tc:
        with tc.tile_pool(name="sb", bufs=1) as sb:
            recg = sb.tile([128, T * D, REC], BF16)
            nc.sync.dma_start(recg[:], d_recg.ap())
            pk = sb.tile([128, bpk], BF16)
            nc.sync.dma_start(pk[:], d_pk.ap())
            xr = pk[:, B_XR:B_XR + T * H].rearrange("p (t h) -> p t h", h=H)
            lmat = pk[:, B_LM:B_LM + T * CP].rearrange("p (t c) -> p t c", c=CP)
            attb = pk[:, B_AT:B_AT + H]
            smask = pk[:, B_SM:B_SM + T * D].rearrange("p (t d) -> p t d", d=D)

            recg4 = recg[:].rearrange("p (t d) r -> p t d r", d=D)
            xr_bc = xr.unsqueeze(2).broadcast_to([128, T, D, H])
            # edge-score chain in bf16 (DVE 2x/4x perf modes)
            h = sb.tile([128, T, D, H], BF16)
            nc.vector.tensor_tensor(h[:], recg4[:, :, :, 0:H], xr_bc, op=ALU.add)
            lr = sb.tile([128, T, D, H], BF16)
            hs = sb.tile([128, T, D, H], BF16)
            nc.scalar.mul(hs[:], h[:], NEG_SLOPE)
            nc.vector.tensor_tensor(lr[:], h[:], hs[:], op=ALU.max)
            ez = sb.tile([128, T, D, H], BF16)
            att_bc = attb.unsqueeze(1).unsqueeze(1).broadcast_to([128, T, D, H])
            nc.vector.tensor_tensor(ez[:], lr[:], att_bc, op=ALU.mult)
            e = sb.tile([128, T, D], F32)
            nc.vector.tensor_reduce(e[:], ez[:], axis=AX.X, op=ALU.add)
            ex = sb.tile([128, T, D], F32)
            nc.scalar.activation(ex[:], e[:], AF.Exp)
            exm = sb.tile([128, T, D], F32)
            nc.vector.tensor_tensor(exm[:], ex[:], smask, op=ALU.mult)
            den = sb.tile([128, T], F32)
            nc.vector.tensor_reduce(den[:], exm[:], axis=AX.X, op=ALU.add)
            nc.vector.tensor_scalar(den[:], den[:], 1e-30, None, op0=ALU.add)
            rden = sb.tile([128, T], F32)
            nc.vector.reciprocal(rden[:], den[:])
            alpha = sb.tile([128, T, D], F32)
            nc.vector.tensor_tensor(
                alpha[:], exm[:], rden[:].unsqueeze(2).broadcast_to([128, T, D]),
                op=ALU.mult,
            )
            kz = sb.tile([128, T, D, C], BF16)
            l_bc = lmat[:, :, 0:C].unsqueeze(2).broadcast_to([128, T, D, C])
            nc.vector.tensor_tensor(kz[:], recg4[:, :, :, H:H + C], l_bc,
                                    op=ALU.mult)
            kv = sb.tile([128, T, D], F32)
            nc.vector.tensor_reduce(kv[:], kz[:], axis=AX.X, op=ALU.add)
            ak = sb.tile([128, T, D], F32)
            nc.vector.tensor_tensor(ak[:], alpha[:], kv[:], op=ALU.mult)
            cev = sb.tile([128, 1], F32)
            nc.vector.tensor_reduce(cev[:], ak[:], axis=AX.XY, op=ALU.add)
            nc.sync.dma_start(d_scal.ap(), cev[:])

    nc.compile()
    return nc


# --------------------------------------------------------------------------
# host-side sharding
# --------------------------------------------------------------------------
def shard_inputs(x_sub, Mu, Var, edge_index_sub, subset_idx,
                 W_param, S_param, W_l, b_l, W_r, b_r, att):
    x_sub = np.asarray(x_sub, np.float32)
    Mu = np.asarray(Mu, np.float32)
    Var = np.asarray(Var, np.float32)
    ei = np.asarray(edge_index_sub).astype(np.int64)
    sub = np.asarray(subset_idx).astype(np.int64)
    W_param = np.asarray(W_param, np.float32)
    S_param = np.asarray(S_param, np.float32)
    W_l = np.asarray(W_l, np.float32)
    W_r = np.asarray(W_r, np.float32)
    b_l = np.asarray(b_l, np.float32)
    b_r = np.asarray(b_r, np.float32)
    att = np.asarray(att, np.float32)

    src, dst = ei[0], ei[1]
    deg = np.bincount(dst, minlength=N)
    D = max(int(deg.max()), 1)

    def rearr(a, w):
        # [GP, w] -> [128, 4*w] in (p, k, c) layout
        return a.reshape(4, 128, w).transpose(1, 0, 2).reshape(128, 4 * w)

    mut = np.zeros((GP, CP), np.float32)
    mut[:G, :C] = Mu.T
    vart = np.ones((GP, CP), np.float32)
    vart[:G, :C] = Var.T
    vart[G, :] = 1e30              # S-row of x^2: killed in A; d patched in
    vart[G + 1, :] = 1e30          # ones-row of x^2 must not contribute
    wl = np.zeros((GP, H), np.float32)
    wl[:G] = W_l
    wl[G + 1] = b_l                # bias via ones-row of x
    wr = np.zeros((GP, H), np.float32)
    wr[:G] = W_r
    wr[G + 1] = b_r
    ident = np.eye(128, dtype=np.float32)

    Wsub_all = W_param[sub]        # host row-gather, per sharding hint
    Ssub_all = S_param[sub, 0]

    in_maps_a = []
    aux = []
    for m in range(NCORES):
        lo, hi = m * NPC, (m + 1) * NPC
        sv = np.zeros(NM, np.float32)
        sv[:NPC] = Ssub_all[lo:hi]
        xtb = np.zeros((GP, NM), np.float32)
        xtb[:G, :NPC] = x_sub[lo:hi].T
        xtb[G, :] = sv                 # S row -> x^2 row 500 = S^2
        xtb[G + 1, :NPC] = 1.0         # ones row (biases)
        wn = np.full((NM, CP), WPAD, np.float32)
        wn[:NPC, :C] = Wsub_all[lo:hi]
        wn[NPC:, :] = 0.0
        wnode = wn.reshape(T, 128, CP).transpose(1, 0, 2).reshape(128, T * CP)
        scol = sv.reshape(T, 128).T
        pk = np.concatenate([
            wnode, scol, ident, rearr(mut, CP), rearr(vart, CP),
            rearr(wl, H), rearr(wr, H),
        ], axis=1).astype(np.float32)
        assert pk.shape[1] == APK
        # edges of this core, slotted by local destination
        sel = (dst >= lo) & (dst < hi)
        sm, dm = src[sel], dst[sel] - lo
        order = np.argsort(dm, kind="stable")
        ds, ss = dm[order], sm[order]
        seg_start = np.searchsorted(ds, ds)
        slot = np.arange(len(ds)) - seg_start
        slot_src = np.zeros((NM, D), np.int64)
        slot_mask = np.zeros((NM, D), np.float32)
        gid = (ss // NPC) * NM + (ss % NPC)   # padded global node id
        slot_src[ds, slot] = gid
        slot_mask[ds, slot] = 1.0
        sgid = slot_src.reshape(T, 128, D).transpose(1, 0, 2).reshape(128, T * D)
        smask = slot_mask.reshape(T, 128, D).transpose(1, 0, 2).reshape(128, T * D)
        in_maps_a.append({
            "xtb": xtb.astype(ml_dtypes.bfloat16),
            "pk": np.ascontiguousarray(pk),
        })
        aux.append({
            "sgid": sgid,
            "smask": np.ascontiguousarray(smask),
            "attb": np.tile(att[None, :], (128, 1)).astype(np.float32),
        })
    return in_maps_a, aux, D


def unshard_p(results_a):
    pparts = []
    for r in results_a:
        p = np.asarray(r["pout"]).reshape(128, T, CP).transpose(1, 0, 2)
        pparts.append(p.reshape(NM, CP)[:NPC, :C])
    return np.concatenate(pparts, axis=0).astype(np.float32)


# --------------------------------------------------------------------------
# entry point
# --------------------------------------------------------------------------
_last_res = None


def kernel(**inputs):
    global _last_res
    in_maps_a, aux, D = shard_inputs(**inputs)
    if "A" not in _prog_cache:
        _prog_cache["A"] = build_program_a()
    if ("B", D) not in _prog_cache:
        _prog_cache[("B", D)] = build_program_b(D)

    res_a = run_bass_kernel_spmd(_prog_cache["A"], in_maps_a,
                                 list(range(NCORES)))
    # exchange: concatenate record tables, gather per-edge-slot records
    rec_all = np.concatenate(
        [np.asarray(r["recout"], np.float32) for r in res_a.results], axis=0
    )
    in_maps_b = []
    for m in range(NCORES):
        recg = rec_all[aux[m]["sgid"].reshape(-1)].reshape(128, T * D * REC)
        pk = np.concatenate([
            np.asarray(res_a.results[m]["xrout"], np.float32),
            np.asarray(res_a.results[m]["lout"], np.float32),
            aux[m]["attb"],
            aux[m]["smask"],
        ], axis=1).astype(np.float32)
        in_maps_b.append({
            "recg": recg.astype(ml_dtypes.bfloat16),
            "pk": np.ascontiguousarray(pk).astype(ml_dtypes.bfloat16),
        })
    res_b = run_bass_kernel_spmd(_prog_cache[("B", D)], in_maps_b,
                                 list(range(NCORES)))
    _last_res = (res_a, res_b)

    ll = sum(float(np.asarray(r["scal"], np.float64).sum()) for r in res_a.results)
    ce = sum(float(np.asarray(r["scal"], np.float64).sum()) for r in res_b.results)
    P = unshard_p(res_a.results)
    return np.float32(ll / N), np.float32(-ce / N), P


if __name__ == "__main__":
    import reference

    inp = reference.setup_inputs()
    inp = {k: np.asarray(v) for k, v in inp.items()}
    out = kernel(**inp)
    print("ll", out[0], "ce", out[1], "P", out[2].shape)
